# revision 9
# baseline (speedup 1.0000x reference)
"""Pairwise cosine-similarity kernel for Trainium2 (8 NeuronCores, SPMD).

Computes out = 16 * normalize(x1) @ normalize(x2).T for x1, x2 [8192, 512] f32.

Sharding: x1 rows are split across the 8 cores (1024 rows each); x2 is
replicated. Each core computes its [1024, 8192] slice of the output; the host
concatenates the slices.

Host-side prep is layout/dtype only: inputs are cast to bf16 and x2 is
additionally shipped pre-transposed ([512, 8192]) so the big operand needs no
on-device transposition. All FLOPs (norms, normalization, GEMM, scaling) run
on device:

  1. x1 (bf16, natural): fused Square+row-sum on ScalarE -> sqrt -> clamp ->
     reciprocal -> x1n = x1 * (16/n1) via per-partition tensor_scalar, then
     PE-transpose (bf16 matmul vs. identity) into x1T [D, rows].
  2. x2 norms from the natural-layout bf16 copy (per column-group of 2048
     rows): Square+row-sum, sqrt, clamp, reciprocal -> inv2 [128, 16] compact.
     PE-transpose inv2 to [16, 128], then broadcast across partitions with
     K=1 ones-matmuls -> inv2_bcast [128, 2048] f32, and scale the
     pre-transposed x2T tiles in place (DVE tensor_tensor).
  3. Main GEMM: out_tile[128, 512] += x1T.T @ x2T over 4 K-chunks (bf16,
     f32 PSUM), PSUM->SBUF copies split across DVE/ACT, DMA out.
"""

import sys

for _p in ("/root/.axon_site/_ro/trn_rl_repo", "/opt/trn_rl_repo"):
    if _p not in sys.path:
        sys.path.append(_p)

import ml_dtypes
import numpy as np

import concourse.bass as bass
import concourse.tile as tile
from concourse import bacc, mybir
from concourse.bass_utils import run_bass_kernel_spmd
from concourse.masks import make_identity

F32 = mybir.dt.float32
BF16 = mybir.dt.bfloat16
P = 128
SCALE = 16.0
EPS = 1e-8

N_CORES = 8
N1 = 8192  # x1 rows (total)
N2 = 8192  # x2 rows
D = 512  # feature dim

_PROGRAM_CACHE = {}


def build_program(n1_local=N1 // N_CORES, n2=N2, d=D, cg_width=2048):
    """Build the SPMD program one core runs. Returns the compiled Bacc.

    DRAM inputs: x1 [n1_local, d] bf16 (natural), x2n [n2, d] bf16 (natural,
    norms only), x2t [d, n2] bf16 (pre-transposed, GEMM operand).
    """
    kc = d // P  # K-chunks of the contraction dim
    m_tiles = n1_local // P  # x1 row-tiles per core
    n_cgs = n2 // cg_width  # output column groups
    nch = cg_width // 512  # 512-wide chunks per column group
    cg_rt = cg_width // P  # x2 row-tiles per column group

    nc = bacc.Bacc("TRN2", target_bir_lowering=False, debug=False,
                   num_devices=N_CORES)
    x1 = nc.dram_tensor("x1", [n1_local, d], BF16, kind="ExternalInput")
    x2n = nc.dram_tensor("x2n", [n2, d], BF16, kind="ExternalInput")
    x2t = nc.dram_tensor("x2t", [d, n2], BF16, kind="ExternalInput")
    out = nc.dram_tensor("out", [n1_local, n2], F32, kind="ExternalOutput")

    with tile.TileContext(nc) as tc:
        with (
            tc.tile_pool(name="const", bufs=1) as const,
            tc.tile_pool(name="ld", bufs=3) as ld,
            tc.tile_pool(name="sq", bufs=3) as sqp,
            tc.tile_pool(name="stat", bufs=4) as stat,
            tc.tile_pool(name="xt", bufs=1) as xt,
            tc.tile_pool(name="bc", bufs=2) as bcp,
            tc.tile_pool(name="outp", bufs=3) as outp,
            tc.tile_pool(name="ps", bufs=6, space="PSUM") as psp,
            tc.tile_pool(name="psb", bufs=2, space="PSUM") as psb,
        ):
            ident_b = const.tile([P, P], BF16)
            make_identity(nc, ident_b)
            ident_f = const.tile([P, P], F32)
            make_identity(nc, ident_f)
            ones128 = const.tile([P, P], F32)
            nc.gpsimd.memset(ones128[:], 1.0)
            ident4 = const.tile([P, 4, P], F32)
            nc.gpsimd.memset(ident4[:], 0.0)
            for b in range(4):
                make_identity(nc, ident4[:, b], nomemset=True)

            x1r = x1.ap().rearrange("(g j p) e -> g p j e", j=4, p=P)
            x2r = x2n.ap().rearrange("(g j p) e -> g p j e", j=4, p=P)

            x1T = [xt.tile([P, n1_local], BF16, tag=f"x1T_{k}", name=f"x1T_{k}")
                   for k in range(kc)]
            x2T = [
                [xt.tile([P, cg_width], BF16, tag=f"x2T_{k}_{cg}",
                         name=f"x2T_{k}_{cg}")
                 for cg in range(n_cgs)]
                for k in range(kc)
            ]

            def row_stats(src_r, g, inv_dst, scale_const):
                """inv_dst [P, 4] = scale / max(row_norm, EPS) for 4 row-tiles."""
                ld_t = ld.tile([P, 4, d], BF16, tag="ld")
                nc.scalar.dma_start(ld_t[:], src_r[g])
                ssq = stat.tile([P, 4], F32, tag="ssq")
                for j in range(4):
                    sq_t = sqp.tile([P, d], BF16, tag="sq")
                    nc.scalar.activation(
                        sq_t[:], ld_t[:, j],
                        mybir.ActivationFunctionType.Square,
                        accum_out=ssq[:, j : j + 1],
                    )
                nrm = stat.tile([P, 4], F32, tag="nrm")
                nc.scalar.activation(
                    nrm[:], ssq[:], mybir.ActivationFunctionType.Sqrt
                )
                nc.vector.tensor_scalar_max(nrm[:], nrm[:], EPS)
                nc.vector.reciprocal(inv_dst, nrm[:])
                if scale_const != 1.0:
                    nc.vector.tensor_scalar_mul(inv_dst, inv_dst, scale_const)
                return ld_t

            # ---- x2 per column group: stats -> bcast -> scale ----------
            def prep_cg(cg):
                for k in range(kc):
                    nc.scalar.dma_start(
                        x2T[k][cg][:],
                        x2t[k * P : (k + 1) * P,
                            cg * cg_width : (cg + 1) * cg_width],
                    )
                # compact inverse norms for the cg's rows: [P, cg_rt]
                inv2 = stat.tile([P, cg_rt], F32, tag="inv2",
                                 name=f"inv2_{cg}")
                for g2 in range(cg_rt // 4):
                    row_stats(x2r, cg * (cg_rt // 4) + g2,
                              inv2[:, g2 * 4 : (g2 + 1) * 4], 1.0)
                # partition-broadcast: bc[:, c*P+p] = inv2[p, c] via
                # ones128.T @ diag(inv2[:, c]) (column sums of a diagonal)
                bc = bcp.tile([P, cg_width], F32, tag="bc", name=f"bc_{cg}")
                for c0 in range(0, cg_rt, 4):
                    dg4 = stat.tile([P, 4, P], F32, tag="dg4",
                                    name=f"dg4_{cg}_{c0}")
                    nc.vector.tensor_mul(
                        dg4[:], ident4[:],
                        inv2[:, c0 : c0 + 4, None].to_broadcast((P, 4, P)),
                    )
                    ps_b = psb.tile([P, 512], F32, tag="psb",
                                    name=f"psb_{cg}_{c0}")
                    nc.tensor.matmul(ps_b[:], lhsT=ones128[:], rhs=dg4[:],
                                     start=True, stop=True)
                    nc.vector.tensor_copy(
                        bc[:, c0 * P : (c0 + 4) * P], ps_b[:]
                    )
                # scale the transposed operand in place (bf16 * f32 -> bf16)
                for k in range(kc):
                    nc.vector.tensor_mul(
                        x2T[k][cg][:], x2T[k][cg][:], bc[:]
                    )

            def gemm_cg(cg):
                for m in range(m_tiles):
                    pss = [psp.tile([P, 512], F32, tag="ps",
                                    name=f"ps_{cg}_{m}_{j}")
                           for j in range(nch)]
                    for k in range(kc):
                        for j in range(nch):
                            nc.tensor.matmul(
                                pss[j][:],
                                lhsT=x1T[k][:, m * P : (m + 1) * P],
                                rhs=x2T[k][cg][:, j * 512 : (j + 1) * 512],
                                start=(k == 0), stop=(k == kc - 1),
                            )
                    ot = outp.tile([P, cg_width], F32, tag="ot",
                                   name=f"ot_{cg}_{m}")
                    for j in range(nch):
                        dst = ot[:, j * 512 : (j + 1) * 512]
                        if j % 2 == 0:
                            nc.vector.tensor_copy(dst, pss[j][:])
                        else:
                            nc.scalar.copy(dst, pss[j][:])
                    nc.sync.dma_start(
                        out[m * P : (m + 1) * P,
                            cg * cg_width : (cg + 1) * cg_width],
                        ot[:],
                    )

            prep_cg(0)
            # ---- x1 (emitted after cg0 prep so ACT/DVE/DMA warm up): stats -> normalize (bf16) -> PE transpose ----------
            for g in range(n1_local // 512):
                inv1 = stat.tile([P, 4], F32, tag="inv1")
                ld_t = row_stats(x1r, g, inv1[:], SCALE)
                x1nrm = sqp.tile([P, 4, d], BF16, tag="x1nrm")
                for j in range(4):
                    nc.vector.tensor_scalar_mul(
                        x1nrm[:, j], ld_t[:, j], inv1[:, j : j + 1]
                    )
                for k in range(kc):
                    ps_t = psb.tile([P, 512], F32, tag="psb")
                    for j in range(4):
                        nc.tensor.matmul(
                            ps_t[:, j * P : (j + 1) * P],
                            lhsT=x1nrm[:, j, k * P : (k + 1) * P],
                            rhs=ident_b[:],
                            start=True, stop=True,
                        )
                    dst = x1T[k][:, g * 512 : (g + 1) * 512]
                    if k % 2 == 0:
                        nc.vector.tensor_copy(dst, ps_t[:])
                    else:
                        nc.scalar.copy(dst, ps_t[:])

            for cg in range(1, n_cgs):
                prep_cg(cg)
                gemm_cg(cg - 1)
            gemm_cg(n_cgs - 1)

    nc.compile()
    return nc


def _get_program():
    key = "default"
    if key not in _PROGRAM_CACHE:
        _PROGRAM_CACHE[key] = build_program()
    return _PROGRAM_CACHE[key]


def make_in_maps(x1: np.ndarray, x2: np.ndarray) -> list:
    x1 = np.asarray(x1, dtype=np.float32)
    x2 = np.asarray(x2, dtype=np.float32)
    assert x1.shape == (N1, D) and x2.shape == (N2, D), (x1.shape, x2.shape)
    x1_b = x1.astype(ml_dtypes.bfloat16)
    x2_b = x2.astype(ml_dtypes.bfloat16)
    x2t_b = np.ascontiguousarray(x2_b.T)
    rows = N1 // N_CORES
    return [
        {
            "x1": np.ascontiguousarray(x1_b[c * rows : (c + 1) * rows]),
            "x2n": x2_b,
            "x2t": x2t_b,
        }
        for c in range(N_CORES)
    ]


def kernel(x1: np.ndarray, x2: np.ndarray) -> np.ndarray:
    nc = _get_program()
    in_maps = make_in_maps(x1, x2)
    res = run_bass_kernel_spmd(nc, in_maps, core_ids=list(range(N_CORES)))
    return np.concatenate([res.results[c]["out"] for c in range(N_CORES)], axis=0)


if __name__ == "__main__":
    rng = np.random.default_rng(0)
    a = rng.standard_normal((N1, D), dtype=np.float32)
    b = rng.standard_normal((N2, D), dtype=np.float32)
    got = kernel(a, b)
    n1 = np.maximum(np.linalg.norm(a, axis=-1, keepdims=True), EPS)
    n2 = np.maximum(np.linalg.norm(b, axis=-1, keepdims=True), EPS)
    want = SCALE * (a / n1) @ (b / n2).T
    err = np.abs(got - want)
    rel = np.linalg.norm(got - want) / np.linalg.norm(want)
    print(f"max abs err: {err.max():.3e}  rel: {rel:.3e}")


# revision 10
# speedup vs baseline: 1.0949x; 1.0949x over previous
"""Pairwise cosine-similarity kernel for Trainium2 (8 NeuronCores, SPMD).

Computes out = 16 * normalize(x1) @ normalize(x2).T for x1, x2 [8192, 512] f32.

Sharding: x1 rows are split across the 8 cores (1024 rows each); x2 is
replicated. Each core computes its [1024, 8192] slice of the output; the host
concatenates the slices.

Host-side prep is layout/dtype only: inputs are cast to bf16 and x2 is
additionally shipped pre-transposed ([512, 8192]) so the big operand needs no
on-device transposition. All FLOPs (norms, normalization, GEMM, scaling) run
on device:

  1. x1 (bf16, natural): fused Square+row-sum on ScalarE -> sqrt -> clamp ->
     reciprocal -> x1n = x1 * (16/n1) via per-partition tensor_scalar, then
     PE-transpose (bf16 matmul vs. identity) into x1T [D, rows].
  2. x2 norms from the natural-layout bf16 copy (per column-group of 2048
     rows): Square+row-sum, sqrt, clamp, reciprocal -> inv2 [128, 16] compact.
     PE-transpose inv2 to [16, 128], then broadcast across partitions with
     K=1 ones-matmuls -> inv2_bcast [128, 2048] f32, and scale the
     pre-transposed x2T tiles in place (DVE tensor_tensor).
  3. Main GEMM: out_tile[128, 512] += x1T.T @ x2T over 4 K-chunks (bf16,
     f32 PSUM), PSUM->SBUF copies split across DVE/ACT, DMA out.
"""

import sys

for _p in ("/root/.axon_site/_ro/trn_rl_repo", "/opt/trn_rl_repo"):
    if _p not in sys.path:
        sys.path.append(_p)

import ml_dtypes
import numpy as np

import concourse.bass as bass
import concourse.tile as tile
from concourse import bacc, mybir
from concourse.bass_utils import run_bass_kernel_spmd
from concourse.masks import make_identity

F32 = mybir.dt.float32
BF16 = mybir.dt.bfloat16
P = 128
SCALE = 16.0
EPS = 1e-8

N_CORES = 8
N1 = 8192  # x1 rows (total)
N2 = 8192  # x2 rows
D = 512  # feature dim

_PROGRAM_CACHE = {}


def build_program(n1_local=N1 // N_CORES, n2=N2, d=D, cg_width=2048):
    """Build the SPMD program one core runs. Returns the compiled Bacc.

    DRAM inputs: x1 [n1_local, d] bf16 (natural), x2n [n2, d] bf16 (natural,
    norms only), x2t [d, n2] bf16 (pre-transposed, GEMM operand).
    """
    kc = d // P  # K-chunks of the contraction dim
    m_tiles = n1_local // P  # x1 row-tiles per core
    n_cgs = n2 // cg_width  # output column groups
    nch = cg_width // 512  # 512-wide chunks per column group
    cg_rt = cg_width // P  # x2 row-tiles per column group

    nc = bacc.Bacc("TRN2", target_bir_lowering=False, debug=False,
                   num_devices=N_CORES)
    x1 = nc.dram_tensor("x1", [n1_local, d], BF16, kind="ExternalInput")
    x2n = nc.dram_tensor("x2n", [n2, d], BF16, kind="ExternalInput")
    x2t = nc.dram_tensor("x2t", [d, n2], BF16, kind="ExternalInput")
    out = nc.dram_tensor("out", [n1_local, n2], F32, kind="ExternalOutput")

    with tile.TileContext(nc) as tc:
        with (
            tc.tile_pool(name="const", bufs=1) as const,
            tc.tile_pool(name="ld", bufs=3) as ld,
            tc.tile_pool(name="sq", bufs=3) as sqp,
            tc.tile_pool(name="stat", bufs=4) as stat,
            tc.tile_pool(name="xt", bufs=1) as xt,
            tc.tile_pool(name="bc", bufs=2) as bcp,
            tc.tile_pool(name="outp", bufs=3) as outp,
            tc.tile_pool(name="ps", bufs=6, space="PSUM") as psp,
            tc.tile_pool(name="psb", bufs=2, space="PSUM") as psb,
        ):
            ident_b = const.tile([P, P], BF16)
            make_identity(nc, ident_b)
            ident_f = const.tile([P, P], F32)
            make_identity(nc, ident_f)
            ones128 = const.tile([P, P], F32)
            nc.gpsimd.memset(ones128[:], 1.0)
            ident4 = const.tile([P, 4, P], F32)
            nc.gpsimd.memset(ident4[:], 0.0)
            for b in range(4):
                make_identity(nc, ident4[:, b], nomemset=True)

            x1r = x1.ap().rearrange("(g j p) e -> g p j e", j=4, p=P)
            x2r = x2n.ap().rearrange("(g j p) e -> g p j e", j=4, p=P)

            x1T = [xt.tile([P, n1_local], BF16, tag=f"x1T_{k}", name=f"x1T_{k}")
                   for k in range(kc)]
            x2T = [
                [xt.tile([P, cg_width], BF16, tag=f"x2T_{k}_{cg}",
                         name=f"x2T_{k}_{cg}")
                 for cg in range(n_cgs)]
                for k in range(kc)
            ]

            def row_stats(src_r, g, inv_dst, scale_const):
                """inv_dst [P, 4] = scale / max(row_norm, EPS) for 4 row-tiles."""
                ld_t = ld.tile([P, 4, d], BF16, tag="ld")
                nc.sync.dma_start(ld_t[:], src_r[g])
                ssq = stat.tile([P, 4], F32, tag="ssq")
                for j in range(4):
                    sq_t = sqp.tile([P, d], BF16, tag="sq")
                    nc.scalar.activation(
                        sq_t[:], ld_t[:, j],
                        mybir.ActivationFunctionType.Square,
                        accum_out=ssq[:, j : j + 1],
                    )
                nrm = stat.tile([P, 4], F32, tag="nrm")
                nc.scalar.activation(
                    nrm[:], ssq[:], mybir.ActivationFunctionType.Sqrt
                )
                nc.vector.tensor_scalar_max(nrm[:], nrm[:], EPS)
                nc.vector.reciprocal(inv_dst, nrm[:])
                if scale_const != 1.0:
                    nc.vector.tensor_scalar_mul(inv_dst, inv_dst, scale_const)
                return ld_t

            # ---- x2 per column group: stats -> bcast -> scale ----------
            def prep_cg(cg):
                for k in range(kc):
                    nc.sync.dma_start(
                        x2T[k][cg][:],
                        x2t[k * P : (k + 1) * P,
                            cg * cg_width : (cg + 1) * cg_width],
                    )
                # compact inverse norms for the cg's rows: [P, cg_rt]
                inv2 = stat.tile([P, cg_rt], F32, tag="inv2",
                                 name=f"inv2_{cg}")
                for g2 in range(cg_rt // 4):
                    row_stats(x2r, cg * (cg_rt // 4) + g2,
                              inv2[:, g2 * 4 : (g2 + 1) * 4], 1.0)
                # partition-broadcast: bc[:, c*P+p] = inv2[p, c] via
                # ones128.T @ diag(inv2[:, c]) (column sums of a diagonal)
                bc = bcp.tile([P, cg_width], F32, tag="bc", name=f"bc_{cg}")
                for c0 in range(0, cg_rt, 4):
                    dg4 = stat.tile([P, 4, P], F32, tag="dg4",
                                    name=f"dg4_{cg}_{c0}")
                    nc.vector.tensor_mul(
                        dg4[:], ident4[:],
                        inv2[:, c0 : c0 + 4, None].to_broadcast((P, 4, P)),
                    )
                    ps_b = psb.tile([P, 512], F32, tag="psb",
                                    name=f"psb_{cg}_{c0}")
                    nc.tensor.matmul(ps_b[:], lhsT=ones128[:], rhs=dg4[:],
                                     start=True, stop=True)
                    nc.vector.tensor_copy(
                        bc[:, c0 * P : (c0 + 4) * P], ps_b[:]
                    )
                # scale the transposed operand in place (bf16 * f32 -> bf16)
                for k in range(kc):
                    nc.vector.tensor_mul(
                        x2T[k][cg][:], x2T[k][cg][:], bc[:]
                    )

            def gemm_cg(cg):
                for m in range(m_tiles):
                    pss = [psp.tile([P, 512], F32, tag="ps",
                                    name=f"ps_{cg}_{m}_{j}")
                           for j in range(nch)]
                    for k in range(kc):
                        for j in range(nch):
                            nc.tensor.matmul(
                                pss[j][:],
                                lhsT=x1T[k][:, m * P : (m + 1) * P],
                                rhs=x2T[k][cg][:, j * 512 : (j + 1) * 512],
                                start=(k == 0), stop=(k == kc - 1),
                            )
                    ot = outp.tile([P, cg_width], F32, tag="ot",
                                   name=f"ot_{cg}_{m}")
                    for j in range(nch):
                        dst = ot[:, j * 512 : (j + 1) * 512]
                        if j % 2 == 0:
                            nc.vector.tensor_copy(dst, pss[j][:])
                        else:
                            nc.scalar.copy(dst, pss[j][:])
                    nc.sync.dma_start(
                        out[m * P : (m + 1) * P,
                            cg * cg_width : (cg + 1) * cg_width],
                        ot[:],
                    )

            # ---- x1 (emitted after cg0 prep so ACT/DVE/DMA warm up): stats -> normalize (bf16) -> PE transpose ----------
            for g in range(n1_local // 512):
                inv1 = stat.tile([P, 4], F32, tag="inv1")
                ld_t = row_stats(x1r, g, inv1[:], SCALE)
                x1nrm = sqp.tile([P, 4, d], BF16, tag="x1nrm")
                for j in range(4):
                    nc.vector.tensor_scalar_mul(
                        x1nrm[:, j], ld_t[:, j], inv1[:, j : j + 1]
                    )
                for k in range(kc):
                    ps_t = psb.tile([P, 512], F32, tag="psb")
                    for j in range(4):
                        nc.tensor.matmul(
                            ps_t[:, j * P : (j + 1) * P],
                            lhsT=x1nrm[:, j, k * P : (k + 1) * P],
                            rhs=ident_b[:],
                            start=True, stop=True,
                        )
                    dst = x1T[k][:, g * 512 : (g + 1) * 512]
                    if k % 2 == 0:
                        nc.vector.tensor_copy(dst, ps_t[:])
                    else:
                        nc.scalar.copy(dst, ps_t[:])

            for cg in range(n_cgs):
                prep_cg(cg)
                gemm_cg(cg)

    nc.compile()
    return nc


def _get_program():
    key = "default"
    if key not in _PROGRAM_CACHE:
        _PROGRAM_CACHE[key] = build_program()
    return _PROGRAM_CACHE[key]


def make_in_maps(x1: np.ndarray, x2: np.ndarray) -> list:
    x1 = np.asarray(x1, dtype=np.float32)
    x2 = np.asarray(x2, dtype=np.float32)
    assert x1.shape == (N1, D) and x2.shape == (N2, D), (x1.shape, x2.shape)
    x1_b = x1.astype(ml_dtypes.bfloat16)
    x2_b = x2.astype(ml_dtypes.bfloat16)
    x2t_b = np.ascontiguousarray(x2_b.T)
    rows = N1 // N_CORES
    return [
        {
            "x1": np.ascontiguousarray(x1_b[c * rows : (c + 1) * rows]),
            "x2n": x2_b,
            "x2t": x2t_b,
        }
        for c in range(N_CORES)
    ]


def kernel(x1: np.ndarray, x2: np.ndarray) -> np.ndarray:
    nc = _get_program()
    in_maps = make_in_maps(x1, x2)
    res = run_bass_kernel_spmd(nc, in_maps, core_ids=list(range(N_CORES)))
    return np.concatenate([res.results[c]["out"] for c in range(N_CORES)], axis=0)


if __name__ == "__main__":
    rng = np.random.default_rng(0)
    a = rng.standard_normal((N1, D), dtype=np.float32)
    b = rng.standard_normal((N2, D), dtype=np.float32)
    got = kernel(a, b)
    n1 = np.maximum(np.linalg.norm(a, axis=-1, keepdims=True), EPS)
    n2 = np.maximum(np.linalg.norm(b, axis=-1, keepdims=True), EPS)
    want = SCALE * (a / n1) @ (b / n2).T
    err = np.abs(got - want)
    rel = np.linalg.norm(got - want) / np.linalg.norm(want)
    print(f"max abs err: {err.max():.3e}  rel: {rel:.3e}")


# revision 11
# speedup vs baseline: 1.1481x; 1.0486x over previous
"""Pairwise cosine-similarity kernel for Trainium2 (8 NeuronCores, SPMD).

Computes out = 16 * normalize(x1) @ normalize(x2).T for x1, x2 [8192, 512] f32.

Sharding: x1 rows are split across the 8 cores (1024 rows each); x2 is
replicated. Each core computes its [1024, 8192] slice of the output; the host
concatenates the slices.

Host-side prep is layout/dtype only: inputs are cast to bf16 and x2 is
additionally shipped pre-transposed ([512, 8192]) so the big operand needs no
on-device transposition. All FLOPs (norms, normalization, GEMM, scaling) run
on device:

  1. x1 (bf16, natural): fused Square+row-sum on ScalarE -> sqrt -> clamp ->
     reciprocal -> x1n = x1 * (16/n1) via per-partition tensor_scalar, then
     PE-transpose (bf16 matmul vs. identity) into x1T [D, rows].
  2. x2 norms from the natural-layout bf16 copy (per column-group of 2048
     rows): Square+row-sum, sqrt, clamp, reciprocal -> inv2 [128, 16] compact.
     PE-transpose inv2 to [16, 128], then broadcast across partitions with
     K=1 ones-matmuls -> inv2_bcast [128, 2048] f32, and scale the
     pre-transposed x2T tiles in place (DVE tensor_tensor).
  3. Main GEMM: out_tile[128, 512] += x1T.T @ x2T over 4 K-chunks (bf16,
     f32 PSUM), PSUM->SBUF copies split across DVE/ACT, DMA out.
"""

import sys

for _p in ("/root/.axon_site/_ro/trn_rl_repo", "/opt/trn_rl_repo"):
    if _p not in sys.path:
        sys.path.append(_p)

import ml_dtypes
import numpy as np

import concourse.bass as bass
import concourse.tile as tile
from concourse import bacc, mybir
from concourse.bass_utils import run_bass_kernel_spmd
from concourse.masks import make_identity

F32 = mybir.dt.float32
BF16 = mybir.dt.bfloat16
P = 128
SCALE = 16.0
EPS = 1e-8

N_CORES = 8
N1 = 8192  # x1 rows (total)
N2 = 8192  # x2 rows
D = 512  # feature dim

_PROGRAM_CACHE = {}


def build_program(n1_local=N1 // N_CORES, n2=N2, d=D, cg_width=1024):
    """Build the SPMD program one core runs. Returns the compiled Bacc.

    DRAM inputs: x1 [n1_local, d] bf16 (natural), x2n [n2, d] bf16 (natural,
    norms only), x2t [d, n2] bf16 (pre-transposed, GEMM operand).
    """
    kc = d // P  # K-chunks of the contraction dim
    m_tiles = n1_local // P  # x1 row-tiles per core
    n_cgs = n2 // cg_width  # output column groups
    nch = cg_width // 512  # 512-wide chunks per column group
    cg_rt = cg_width // P  # x2 row-tiles per column group

    nc = bacc.Bacc("TRN2", target_bir_lowering=False, debug=False,
                   num_devices=N_CORES)
    x1 = nc.dram_tensor("x1", [n1_local, d], BF16, kind="ExternalInput")
    x2n = nc.dram_tensor("x2n", [n2, d], BF16, kind="ExternalInput")
    x2t = nc.dram_tensor("x2t", [d, n2], BF16, kind="ExternalInput")
    out = nc.dram_tensor("out", [n1_local, n2], F32, kind="ExternalOutput")

    with tile.TileContext(nc) as tc:
        with (
            tc.tile_pool(name="const", bufs=1) as const,
            tc.tile_pool(name="ld", bufs=3) as ld,
            tc.tile_pool(name="sq", bufs=3) as sqp,
            tc.tile_pool(name="stat", bufs=4) as stat,
            tc.tile_pool(name="xt", bufs=1) as xt,
            tc.tile_pool(name="bc", bufs=2) as bcp,
            tc.tile_pool(name="outp", bufs=3) as outp,
            tc.tile_pool(name="ps", bufs=6, space="PSUM") as psp,
            tc.tile_pool(name="psb", bufs=2, space="PSUM") as psb,
        ):
            ident_b = const.tile([P, P], BF16)
            make_identity(nc, ident_b)
            ident_f = const.tile([P, P], F32)
            make_identity(nc, ident_f)
            ones128 = const.tile([P, P], F32)
            nc.gpsimd.memset(ones128[:], 1.0)
            ident4 = const.tile([P, 4, P], F32)
            nc.gpsimd.memset(ident4[:], 0.0)
            for b in range(4):
                make_identity(nc, ident4[:, b], nomemset=True)

            x1r = x1.ap().rearrange("(g j p) e -> g p j e", j=4, p=P)
            x2r = x2n.ap().rearrange("(g j p) e -> g p j e", j=4, p=P)

            x1T = [xt.tile([P, n1_local], BF16, tag=f"x1T_{k}", name=f"x1T_{k}")
                   for k in range(kc)]
            x2T = [
                [xt.tile([P, cg_width], BF16, tag=f"x2T_{k}_{cg}",
                         name=f"x2T_{k}_{cg}")
                 for cg in range(n_cgs)]
                for k in range(kc)
            ]

            def row_stats(src_r, g, inv_dst, scale_const):
                """inv_dst [P, 4] = scale / max(row_norm, EPS) for 4 row-tiles."""
                ld_t = ld.tile([P, 4, d], BF16, tag="ld")
                nc.sync.dma_start(ld_t[:], src_r[g])
                ssq = stat.tile([P, 4], F32, tag="ssq")
                for j in range(4):
                    sq_t = sqp.tile([P, d], BF16, tag="sq")
                    nc.scalar.activation(
                        sq_t[:], ld_t[:, j],
                        mybir.ActivationFunctionType.Square,
                        accum_out=ssq[:, j : j + 1],
                    )
                nrm = stat.tile([P, 4], F32, tag="nrm")
                nc.scalar.activation(
                    nrm[:], ssq[:], mybir.ActivationFunctionType.Sqrt
                )
                nc.vector.tensor_scalar_max(nrm[:], nrm[:], EPS)
                nc.vector.reciprocal(inv_dst, nrm[:])
                if scale_const != 1.0:
                    nc.vector.tensor_scalar_mul(inv_dst, inv_dst, scale_const)
                return ld_t

            # ---- x2 per column group: stats -> bcast -> scale ----------
            def prep_cg(cg):
                for k in range(kc):
                    nc.sync.dma_start(
                        x2T[k][cg][:],
                        x2t[k * P : (k + 1) * P,
                            cg * cg_width : (cg + 1) * cg_width],
                    )
                # compact inverse norms for the cg's rows: [P, cg_rt]
                inv2 = stat.tile([P, cg_rt], F32, tag="inv2",
                                 name=f"inv2_{cg}")
                for g2 in range(cg_rt // 4):
                    row_stats(x2r, cg * (cg_rt // 4) + g2,
                              inv2[:, g2 * 4 : (g2 + 1) * 4], 1.0)
                # partition-broadcast: bc[:, c*P+p] = inv2[p, c] via
                # ones128.T @ diag(inv2[:, c]) (column sums of a diagonal)
                bc = bcp.tile([P, cg_width], F32, tag="bc", name=f"bc_{cg}")
                for c0 in range(0, cg_rt, 4):
                    dg4 = stat.tile([P, 4, P], F32, tag="dg4",
                                    name=f"dg4_{cg}_{c0}")
                    nc.vector.tensor_mul(
                        dg4[:], ident4[:],
                        inv2[:, c0 : c0 + 4, None].to_broadcast((P, 4, P)),
                    )
                    ps_b = psb.tile([P, 512], F32, tag="psb",
                                    name=f"psb_{cg}_{c0}")
                    nc.tensor.matmul(ps_b[:], lhsT=ones128[:], rhs=dg4[:],
                                     start=True, stop=True)
                    nc.vector.tensor_copy(
                        bc[:, c0 * P : (c0 + 4) * P], ps_b[:]
                    )
                # scale the transposed operand in place (bf16 * f32 -> bf16)
                for k in range(kc):
                    nc.vector.tensor_mul(
                        x2T[k][cg][:], x2T[k][cg][:], bc[:]
                    )

            def gemm_cg(cg):
                for m in range(m_tiles):
                    pss = [psp.tile([P, 512], F32, tag="ps",
                                    name=f"ps_{cg}_{m}_{j}")
                           for j in range(nch)]
                    for k in range(kc):
                        for j in range(nch):
                            nc.tensor.matmul(
                                pss[j][:],
                                lhsT=x1T[k][:, m * P : (m + 1) * P],
                                rhs=x2T[k][cg][:, j * 512 : (j + 1) * 512],
                                start=(k == 0), stop=(k == kc - 1),
                            )
                    ot = outp.tile([P, cg_width], F32, tag="ot",
                                   name=f"ot_{cg}_{m}")
                    for j in range(nch):
                        dst = ot[:, j * 512 : (j + 1) * 512]
                        if j % 2 == 0:
                            nc.vector.tensor_copy(dst, pss[j][:])
                        else:
                            nc.scalar.copy(dst, pss[j][:])
                    nc.sync.dma_start(
                        out[m * P : (m + 1) * P,
                            cg * cg_width : (cg + 1) * cg_width],
                        ot[:],
                    )

            # ---- x1 (emitted after cg0 prep so ACT/DVE/DMA warm up): stats -> normalize (bf16) -> PE transpose ----------
            for g in range(n1_local // 512):
                inv1 = stat.tile([P, 4], F32, tag="inv1")
                ld_t = row_stats(x1r, g, inv1[:], SCALE)
                x1nrm = sqp.tile([P, 4, d], BF16, tag="x1nrm")
                for j in range(4):
                    nc.vector.tensor_scalar_mul(
                        x1nrm[:, j], ld_t[:, j], inv1[:, j : j + 1]
                    )
                for k in range(kc):
                    ps_t = psb.tile([P, 512], F32, tag="psb")
                    for j in range(4):
                        nc.tensor.matmul(
                            ps_t[:, j * P : (j + 1) * P],
                            lhsT=x1nrm[:, j, k * P : (k + 1) * P],
                            rhs=ident_b[:],
                            start=True, stop=True,
                        )
                    dst = x1T[k][:, g * 512 : (g + 1) * 512]
                    if k % 2 == 0:
                        nc.vector.tensor_copy(dst, ps_t[:])
                    else:
                        nc.scalar.copy(dst, ps_t[:])

            for cg in range(n_cgs):
                prep_cg(cg)
                gemm_cg(cg)

    nc.compile()
    return nc


def _get_program():
    key = "default"
    if key not in _PROGRAM_CACHE:
        _PROGRAM_CACHE[key] = build_program()
    return _PROGRAM_CACHE[key]


def make_in_maps(x1: np.ndarray, x2: np.ndarray) -> list:
    x1 = np.asarray(x1, dtype=np.float32)
    x2 = np.asarray(x2, dtype=np.float32)
    assert x1.shape == (N1, D) and x2.shape == (N2, D), (x1.shape, x2.shape)
    x1_b = x1.astype(ml_dtypes.bfloat16)
    x2_b = x2.astype(ml_dtypes.bfloat16)
    x2t_b = np.ascontiguousarray(x2_b.T)
    rows = N1 // N_CORES
    return [
        {
            "x1": np.ascontiguousarray(x1_b[c * rows : (c + 1) * rows]),
            "x2n": x2_b,
            "x2t": x2t_b,
        }
        for c in range(N_CORES)
    ]


def kernel(x1: np.ndarray, x2: np.ndarray) -> np.ndarray:
    nc = _get_program()
    in_maps = make_in_maps(x1, x2)
    res = run_bass_kernel_spmd(nc, in_maps, core_ids=list(range(N_CORES)))
    return np.concatenate([res.results[c]["out"] for c in range(N_CORES)], axis=0)


if __name__ == "__main__":
    rng = np.random.default_rng(0)
    a = rng.standard_normal((N1, D), dtype=np.float32)
    b = rng.standard_normal((N2, D), dtype=np.float32)
    got = kernel(a, b)
    n1 = np.maximum(np.linalg.norm(a, axis=-1, keepdims=True), EPS)
    n2 = np.maximum(np.linalg.norm(b, axis=-1, keepdims=True), EPS)
    want = SCALE * (a / n1) @ (b / n2).T
    err = np.abs(got - want)
    rel = np.linalg.norm(got - want) / np.linalg.norm(want)
    print(f"max abs err: {err.max():.3e}  rel: {rel:.3e}")
